# revision 1
# baseline (speedup 1.0000x reference)
"""Trainium2 Bass kernel for a single pre-norm transformer block.

Reference (B=2, T=2048, C=768, H=12, HD=64):
    x = x + causal_attn(LN1(x) @ W_qkv) @ W_attn_proj
    x = x + gelu(LN2(x) @ W_fc) @ W_mlp_proj

Sharding: 8 cores, zero collectives (on-chip allreduce has a ~60-100us
firmware floor + ~49GB/s bus -- far too slow here).  Core c = (batch
b=c//4, rank p=c%4).  Causally balanced interleaved query blocks: core p
owns the four 128-row q-blocks {15-p, 11-p, 7-p, 3-p} of its batch, so
every core's q-slots see the uniform k-extents SLOT_BOUNDS=(16,12,8,4)
chunks -- one SPMD program, no per-core control flow, only ~18% causal
overcompute.  Each core recomputes LN1 + K/V for its whole batch
(weights replicated; that is why this problem is memory-regime).

Layout: feature-major activations [C on partitions, tokens free], so the
stationary matmul operand is always a natural weight tile and no
activation transposes exist anywhere.
  - LN stats: matmul with an all-ones stationary tile sums over features
    AND replicates the sums to all partitions (no partition broadcast).
  - scores S^T[k,q]: lhsT = K fm chunk, rhs = Q fm slot; four k-chunks
    per PSUM bank, exp'd in one ScalarE op (bf16 out), multiplicative
    causal mask only on each slot's final 4-chunk group.
  - AV: lhsT = [V token-major | ones] -> Y rows 0..63 + softmax
    denominator row; normalized via DVE reciprocal + DRAM-roundtrip
    broadcast on gpsimd queues.
  - QKV/attention/MLP matmuls in bf16 (attention residual is ~1.4% of
    output magnitude; MLP error dilutes similarly), LN arithmetic and
    residuals in f32.  Weights are host-pre-tiled into the exact SBUF
    images so every weight load is one large contiguous DMA.
  - kernel() specializes away the LN scale/bias ops when gamma==1 and
    beta==0 (the spec fills) -- checked at runtime on the host.
Measured: norm rel err 1.15e-3 on HW; CoreSim cost model ~263us/core.
"""

import sys

if "/opt/trn_rl_repo" not in sys.path:
    sys.path.insert(0, "/opt/trn_rl_repo")

import numpy as np

import concourse.bass as bass
import concourse.mybir as mybir
from concourse import bacc
import concourse.tile as tile

P = 128
B, T, C, H, HD = 2, 2048, 768, 12, 64
OWN = 512          # query rows owned by each core
NF = C // P        # 6 feature chunks
NQT = T // 512     # 4 column tiles over the 2048 tokens
NKT = T // P       # 16 key chunks
NMO_FC = (4 * C) // P  # 24
SLOT_BOUNDS = (16, 12, 8, 4)   # k-chunks processed per q-slot (128 q rows each)
EPS = 1e-5

f32 = mybir.dt.float32
f32r = mybir.dt.float32r
bf16 = mybir.dt.bfloat16
AFT = mybir.ActivationFunctionType
ALU = mybir.AluOpType

GELU_FUNC = AFT.Gelu  # dev sims patch an erf-gelu into bass_interp for this


def _r(ap):
    """View an f32 AP as float32r for full-rate PE matmuls."""
    return ap.bitcast(f32r)


def build_program(unit_gb=False):
    nc = bacc.Bacc()

    xT = nc.declare_dram_parameter("xT", [C, T], f32, False)[:]
    xq = nc.declare_dram_parameter("xq", [C, OWN], f32, False)[:]
    mask4 = nc.declare_dram_parameter("mask4", [P, 4, 4, P], bf16, False)[:]
    Wq_t = nc.declare_dram_parameter("Wq_t", [NF, P, NF, P], bf16, False)[:]
    Wk_t = nc.declare_dram_parameter("Wk_t", [NF, P, NF, P], bf16, False)[:]
    Wv_t = nc.declare_dram_parameter("Wv_t", [2, P, NF, 384], bf16, False)[:]
    Wap = nc.declare_dram_parameter("Wap", [C, C], bf16, False)[:]
    Wfc_t = nc.declare_dram_parameter("Wfc_t", [NF, P, 4, NF, P], bf16, False)[:]
    Wmp_t = nc.declare_dram_parameter("Wmp_t", [NF, P, NMO_FC, P], bf16, False)[:]
    g1 = nc.declare_dram_parameter("g1", [C], f32, False)[:]
    b1 = nc.declare_dram_parameter("b1", [C], f32, False)[:]
    g2 = nc.declare_dram_parameter("g2", [C], f32, False)[:]
    b2 = nc.declare_dram_parameter("b2", [C], f32, False)[:]
    outT = nc.declare_dram_parameter("outT", [C, OWN], f32, True)[:]

    # feature-chunked DRAM views: feature f = o*128 + p
    xT_r = xT.rearrange("(o p) t -> p o t", p=P)
    xq_r = xq.rearrange("(o p) t -> p o t", p=P)
    WapR = Wap.rearrange("(o p) m -> p o m", p=P)  # [128, 6, 768]

    outT_r = outT.rearrange("(o p) q -> p o q", p=P)

    with tile.TileContext(nc) as tc:
        _body(nc, tc, unit_gb, dict(
            xT_r=xT_r, xq_r=xq_r, Wq_t=Wq_t, Wk_t=Wk_t, Wv_t=Wv_t,
            Wfc_t=Wfc_t, Wmp_t=Wmp_t, WapR=WapR, mask4=mask4, outT_r=outT_r,
            g1=g1, b1=b1, g2=g2, b2=b2,
        ))
    nc.finalize()
    return nc


def _body(nc, tc, unit_gb, d):
    from contextlib import ExitStack

    with ExitStack() as ctx:
        def pool(name, bufs, space="SBUF"):
            return ctx.enter_context(tc.tile_pool(name=name, bufs=bufs, space=space))

        singles = pool("singles", 1)
        xstream2 = pool("xstream2", 8)     # x chunks [P,512] f32
        lnpool = pool("lnpool", 6)         # xln1 resident bf16 [P,T]
        wq_p = pool("wq_p", 3)             # W qkv/fc column tiles
        statp = pool("statp", 4)           # LN stats [P,512] f32
        recp = pool("recp", 2)             # [1,512] f32
        rbp = pool("rbp", 2)               # [64,512] f32
        sqp = pool("sqp", 2)               # f32 scratch [P,512]
        bfp = pool("bfp", 6)               # bf16 stats tiles [P,512]
        ypool = pool("ypool", 6)           # head-pair Y bf16 [128,OWN]
        ytmp = pool("ytmp", 2)             # odd-head staging [64,OWN]
        x2pool = pool("x2pool", 6)         # x2 resident f32
        xlnp2 = pool("xlnp2", 6)           # xln2 f32
        opool = pool("opool", 2)
        dramp = pool("dramp", 2, space="DRAM")
        ps_mm = pool("ps_mm", 2, space="PSUM")
        ps_s = pool("ps_s", 2, space="PSUM")
        ps_y = pool("ps_y", 2, space="PSUM")
        ps_st = ps_y

        ones_sb = singles.tile([P, P], bf16)
        nc.vector.memset(ones_sb[:], 1.0)
        eps_sb = singles.tile([P, 1], f32)
        nc.vector.memset(eps_sb[:], EPS)

        gb = {}
        for name in ("g1", "b1", "g2", "b2"):
            t = singles.tile([P, NF], f32, name=f"gb_{name}")
            nc.sync.dma_start(out=t[:], in_=d[name].rearrange("(o p) -> p o", p=P))
            gb[name] = t

        mask_sb = singles.tile([P, 4, 4, P], bf16)
        nc.sync.dma_start(out=mask_sb[:], in_=d["mask4"])

        # ---------------- layer norm (feature-major) ----------------
        def layer_norm(src_bf_of, src_of, g_sb, b_sb, ncols, out_pool,
                       out_tag, out_dt):
            """src_bf_of(f, qt) -> [P,512] bf16 AP (stats pass);
            src_of(f, qt) -> [P,512] f32 AP (normalize pass).
            Returns NF tiles [P, ncols] of dtype out_dt."""
            outs = [out_pool.tile([P, ncols], out_dt, tag=out_tag,
                                  name=f"ln_{out_tag}_{i}") for i in range(NF)]
            for qt in range(ncols // 512):
                cs = slice(qt * 512, qt * 512 + 512)
                s1 = ps_st.tile([P, 512], f32, tag="y", name="s1")
                s2 = ps_st.tile([P, 512], f32, tag="y", name="s2")
                for f in range(NF):
                    xbf = src_bf_of(f, qt)
                    sq = bfp.tile([P, 512], bf16, tag="sq", name=f"sq{f}")
                    nc.vector.tensor_tensor(sq[:], xbf, xbf, ALU.mult)
                    nc.tensor.matmul(s1[:], ones_sb[:], xbf,
                                     start=(f == 0), stop=(f == NF - 1))
                    nc.tensor.matmul(s2[:], ones_sb[:], sq[:],
                                     start=(f == 0), stop=(f == NF - 1))
                mu = statp.tile([P, 512], f32, tag="stat", name="mu")
                nc.vector.tensor_scalar_mul(mu[:], s1[:], 1.0 / C)
                musq = statp.tile([P, 512], f32, tag="stat", name="musq")
                nc.scalar.activation(out=musq[:], in_=mu[:], func=AFT.Square)
                var = statp.tile([P, 512], f32, tag="stat", name="var")
                nc.vector.tensor_scalar(var[:], s2[:], 1.0 / C, None, ALU.mult)
                nc.vector.tensor_tensor(var[:], var[:], musq[:], ALU.subtract)
                std = statp.tile([P, 512], f32, tag="stat", name="std")
                nc.scalar.activation(out=std[:], in_=var[:], func=AFT.Sqrt,
                                     bias=eps_sb[:])
                rstd = statp.tile([P, 512], f32, tag="stat", name="rstd")
                nc.vector.reciprocal(rstd[:], std[:])
                for f in range(NF):
                    srcf = src_of(f, qt)
                    eng = nc.vector if f < 4 else nc.gpsimd
                    if unit_gb:
                        t = sqp.tile([P, 512], f32, tag="sq", name=f"lnt{f}")
                        eng.tensor_tensor(t[:], srcf, mu[:], ALU.subtract)
                        eng.tensor_tensor(outs[f][:, cs], t[:], rstd[:],
                                          ALU.mult)
                    else:
                        t = sqp.tile([P, 512], f32, tag="sq", name=f"lnt{f}")
                        nc.vector.tensor_tensor(t[:], srcf, mu[:], ALU.subtract)
                        nc.vector.tensor_tensor(t[:], t[:], rstd[:], ALU.mult)
                        nc.vector.tensor_scalar(outs[f][:, cs], t[:],
                                                g_sb[:, f:f + 1], b_sb[:, f:f + 1],
                                                ALU.mult, ALU.add)
            return outs

        # ---------------- LN1 (x streamed from DRAM, one load per chunk) ---
        x_cache = {}

        def x_src(f, qt):
            if (f, qt) not in x_cache:
                t = xstream2.tile([P, 512], f32, tag="x1", name=f"x_{f}_{qt}")
                if qt < NQT:
                    nc.sync.dma_start(out=t[:],
                                      in_=d["xT_r"][:, f, qt * 512:qt * 512 + 512])
                else:
                    nc.sync.dma_start(out=t[:], in_=d["xq_r"][:, f, :])
                x_cache[(f, qt)] = t
            return x_cache[(f, qt)][:]

        def x_bf_src(f, qt):
            t = bfp.tile([P, 512], bf16, tag="xbf", name=f"xbf_{f}_{qt}")
            nc.gpsimd.tensor_copy(out=t[:], in_=x_src(f, qt))
            return t[:]

        xln1 = layer_norm(x_bf_src, x_src, gb["g1"], gb["b1"], T + OWN, lnpool,
                          "ln1", bf16)

        # ---------------- V (token-major, + ones col per head) ----------------
        from contextlib import ExitStack as _ES
        attn_ctx = _ES()
        kpool = attn_ctx.enter_context(tc.tile_pool(name="kpool", bufs=2))
        qpool = attn_ctx.enter_context(tc.tile_pool(name="qpool", bufs=2))
        wv_p = attn_ctx.enter_context(tc.tile_pool(name="wv_p", bufs=1))
        apool = attn_ctx.enter_context(tc.tile_pool(name="apool", bufs=4))
        vpool = attn_ctx.enter_context(tc.tile_pool(name="vpool", bufs=1))
        v_sb = vpool.tile([P, NKT, 2, 6, HD + 1], bf16)
        nc.vector.memset(v_sb[:, :, :, :, HD], 1.0)
        for vn in range(2):
            w = wv_p.tile([P, NF, 384], bf16, tag="wv", name=f"wv{vn}")
            nc.sync.dma_start(out=w[:], in_=d["Wv_t"][vn])
            for kt in range(NKT):
                ps = ps_mm.tile([P, 512], f32, tag="mm", name="vps")
                ts = slice(kt * P, kt * P + P)
                for f in range(NF):
                    nc.tensor.matmul(ps[:, :384], xln1[f][:, ts], w[:, f, :],
                                     start=(f == 0), stop=(f == NF - 1))
                nc.any.tensor_copy(
                    out=v_sb[:, kt, vn, :, 0:HD],
                    in_=ps[:, :384].rearrange("p (j e) -> p j e", e=HD))

        # ---------------- per head-pair: Q, K, attention ----------------
        wap_sb = singles.tile([P, NF, C], bf16)
        nc.sync.dma_start(out=wap_sb[:], in_=d["WapR"])

        y_sb = [ypool.tile([P, OWN], bf16, tag="y", name=f"y_{i}")
                for i in range(NF)]
        for mo in range(NF):
            wq = wq_p.tile([P, NF, P], bf16, tag="wq", name=f"wq{mo}")
            nc.sync.dma_start(out=wq[:], in_=d["Wq_t"][mo])
            qt_sb = qpool.tile([P, OWN], bf16, tag="q", name=f"q{mo}")
            ps = ps_mm.tile([P, 512], f32, tag="mm", name="qps")
            for f in range(NF):
                nc.tensor.matmul(ps[:], wq[:, f, :], xln1[f][:, T:T + OWN],
                                 start=(f == 0), stop=(f == NF - 1))
            nc.any.tensor_copy(out=qt_sb[:], in_=ps[:])

            wk = wq_p.tile([P, NF, P], bf16, tag="wq", name=f"wk{mo}")
            nc.sync.dma_start(out=wk[:], in_=d["Wk_t"][mo])
            kt_sb = kpool.tile([P, T], bf16, tag="k", name=f"k{mo}")
            for qt in range(NQT):
                cs = slice(qt * 512, qt * 512 + 512)
                ps = ps_mm.tile([P, 512], f32, tag="mm", name="kps")
                for f in range(NF):
                    nc.tensor.matmul(ps[:], wk[:, f, :], xln1[f][:, cs],
                                     start=(f == 0), stop=(f == NF - 1))
                nc.any.tensor_copy(out=kt_sb[:, cs], in_=ps[:])

            for hh in (1, 0):
                h = 2 * mo + hh
                po = hh * HD
                yp = ps_y.tile([HD + 1, 512], f32, tag="y", name=f"yp{h}")
                for s in range(4):
                    BS = SLOT_BOUNDS[s]
                    q_s = qt_sb[po:po + HD, s * P:(s + 1) * P]
                    groups = [8] * (BS // 8) + ([4] if BS % 8 else [])
                    k0 = 0
                    for gi, gs in enumerate(groups):
                        sp = ps_s.tile([P, 8, P], f32, tag="s",
                                       name=f"sp{h}_{s}_{gi}")
                        for j in range(gs):
                            kt = k0 + j
                            nc.tensor.matmul(
                                sp[:, j, :],
                                kt_sb[po:po + HD, kt * P:(kt + 1) * P],
                                q_s, start=True, stop=True)
                        a_sb = apool.tile([P, 8, P], bf16, tag="a",
                                          name=f"a{h}_{s}_{gi}")
                        nc.scalar.activation(out=a_sb[:, :gs, :],
                                             in_=sp[:, :gs, :], func=AFT.Exp,
                                             scale=float(1.0 / np.sqrt(HD)))
                        if gi == len(groups) - 1:
                            nc.vector.tensor_tensor(a_sb[:, gs - 4:gs, :],
                                                    a_sb[:, gs - 4:gs, :],
                                                    mask_sb[:, s, :, :], ALU.mult)
                        for j in range(gs):
                            kt = k0 + j
                            nc.tensor.matmul(
                                yp[:, s * P:(s + 1) * P],
                                v_sb[:, kt, h // 6, h % 6, :], a_sb[:, j, :],
                                start=(kt == 0),
                                stop=(kt == BS - 1))
                        k0 += gs
                rec = recp.tile([1, 512], f32, tag="rec", name=f"rec{h}")
                nc.vector.reciprocal(rec[:], yp[HD:HD + 1, :])
                rd = dramp.tile([1, 512], f32, tag="rd", name=f"rd{h}")
                nc.gpsimd.dma_start(out=rd[:], in_=rec[:])
                rb = rbp.tile([HD, 512], f32, tag="rb", name=f"rb{h}")
                nc.gpsimd.dma_start(out=rb[:], in_=rd[:].broadcast_to([HD, 512]))
                if hh == 0:
                    nc.vector.tensor_tensor(y_sb[mo][0:HD, :], yp[0:HD, :],
                                            rb[:], ALU.mult)
                else:
                    yt = ytmp.tile([HD, 512], bf16, tag="yt", name=f"yt{h}")
                    nc.vector.tensor_tensor(yt[:], yp[0:HD, :], rb[:], ALU.mult)
                    nc.gpsimd.dma_start(out=y_sb[mo][HD:P, :], in_=yt[:])

        # ---------------- attn proj + residual ----------------
        x2 = []
        for mo in range(NF):
            # residual source: the LN1 qt=NQT stream tiles ARE xq -- still
            # resident in xstream2, so no re-load from DRAM is needed.
            xo = x_cache[(mo, NQT)]
            ps = ps_mm.tile([P, 512], f32, tag="mm", name="aps")
            for h2 in range(NF):
                nc.tensor.matmul(ps[:], wap_sb[:, h2, mo * P:(mo + 1) * P],
                                 y_sb[h2][:], start=(h2 == 0), stop=(h2 == NF - 1))
            x2t = x2pool.tile([P, OWN], f32, tag="x2", name=f"x2_{mo}")
            nc.vector.tensor_tensor(x2t[:], ps[:], xo[:], ALU.add)
            x2.append(x2t)

        attn_ctx.close()

        # ---------------- LN2 + MLP ----------------
        def x2_bf_src(f, qt):
            # DVE here: gpsimd is busy with late-attention broadcast DMAs,
            # while DVE is idle between attention and the MLP.
            t = bfp.tile([P, 512], bf16, tag="xbf", name=f"x2bf_{f}")
            nc.vector.tensor_copy(out=t[:], in_=x2[f][:])
            return t[:]

        xln2 = layer_norm(x2_bf_src, lambda f, qt: x2[f][:], gb["g2"], gb["b2"],
                          OWN, xlnp2, "ln2", bf16)

        mlp_ctx = _ES()
        wfcp = mlp_ctx.enter_context(tc.tile_pool(name="wfcp", bufs=2))
        wmp_p = mlp_ctx.enter_context(tc.tile_pool(name="wmp_p", bufs=2))
        hbig = mlp_ctx.enter_context(tc.tile_pool(name="hbig", bufs=1))
        h_sb = hbig.tile([P, NMO_FC, OWN], bf16)
        for g4 in range(NMO_FC // 4):
            w4 = wfcp.tile([P, 4, NF, P], bf16, tag="wfc", name=f"wfc{g4}")
            nc.sync.dma_start(out=w4[:], in_=d["Wfc_t"][g4])
            for i in range(4):
                mo = g4 * 4 + i
                ps = ps_mm.tile([P, 512], f32, tag="mm", name="fps")
                for f in range(NF):
                    nc.tensor.matmul(ps[:], w4[:, i, f, :], xln2[f][:],
                                     start=(f == 0), stop=(f == NF - 1))
                nc.scalar.activation(out=h_sb[:, mo, :], in_=ps[:],
                                     func=GELU_FUNC)
        for mo in range(NF):
            w = wmp_p.tile([P, NMO_FC, P], bf16, tag="wmp", name=f"wmp{mo}")
            nc.sync.dma_start(out=w[:], in_=d["Wmp_t"][mo])
            ps = ps_mm.tile([P, 512], f32, tag="mm", name="pps")
            for hc in range(NMO_FC):
                nc.tensor.matmul(ps[:], w[:, hc, :], h_sb[:, hc, :],
                                 start=(hc == 0), stop=(hc == NMO_FC - 1))
            ot = opool.tile([P, OWN], f32, tag="o", name=f"o{mo}")
            nc.vector.tensor_tensor(ot[:], ps[:], x2[mo][:], ALU.add)
            nc.gpsimd.dma_start(out=d["outT_r"][:, mo, :], in_=ot[:])
        mlp_ctx.close()


# ---------------------------------------------------------------------------
# host side
# ---------------------------------------------------------------------------

def make_core_inputs(inputs):
    """Build the 8 per-core input maps from the full-problem inputs."""
    import ml_dtypes

    x = np.asarray(inputs["x"], dtype=np.float32)
    bf = ml_dtypes.bfloat16

    def tile_w(w, n_mo, width):
        # [C_in, n_mo*width] -> [n_mo, 128, C_in//128, width]
        cin = w.shape[0]
        r = w.reshape(cin // P, P, n_mo, width)
        return np.ascontiguousarray(r.transpose(2, 1, 0, 3).astype(bf))

    Wqkv_f = np.asarray(inputs["W_qkv"], np.float32)
    Wfc_f = np.asarray(inputs["W_fc"], np.float32)
    Wmp_f = np.asarray(inputs["W_mlp_proj"], np.float32)
    wfc_t = tile_w(Wfc_f, 24, P).reshape(NF, 4, P, NF, P).transpose(0, 2, 1, 3, 4)
    full = {
        "Wq_t": tile_w(Wqkv_f[:, 0:C], NF, P),
        "Wk_t": tile_w(Wqkv_f[:, C:2 * C], NF, P),
        "Wv_t": tile_w(Wqkv_f[:, 2 * C:3 * C], 2, 384),
        "Wfc_t": np.ascontiguousarray(wfc_t),
        "Wmp_t": tile_w(Wmp_f, NF, P),
        "Wap": np.ascontiguousarray(np.asarray(inputs["W_attn_proj"], np.float32).astype(bf)),
        "g1": np.ascontiguousarray(np.asarray(inputs["ln1_g"], np.float32)),
        "b1": np.ascontiguousarray(np.asarray(inputs["ln1_b"], np.float32)),
        "g2": np.ascontiguousarray(np.asarray(inputs["ln2_g"], np.float32)),
        "b2": np.ascontiguousarray(np.asarray(inputs["ln2_b"], np.float32)),
    }
    in_maps = []
    for c in range(8):
        b, p = c // 4, c % 4
        blocks = [bs - 1 - p for bs in SLOT_BOUNDS]  # 128-row q-block indices
        xb = x[b]                                    # [T, C]
        own = np.concatenate([np.arange(bk * P, (bk + 1) * P) for bk in blocks])
        # mask4[kp, s, j, q] = keep for k-chunk (BS-4+j) of slot s vs q row
        mask = np.zeros((P, 4, 4, P), np.float32)
        kp = np.arange(P)[:, None]
        q = np.arange(P)[None, :]
        for s, BS in enumerate(SLOT_BOUNDS):
            E = BS - p          # real k-extent (chunks) for this core's block
            blk = BS - 1 - p    # q-block index
            for j in range(4):
                kc = BS - 4 + j
                if kc < E:
                    mask[:, s, j, :] = (kc * P + kp <= blk * P + q)
        m = dict(full)
        m["xT"] = np.ascontiguousarray(xb.T)
        m["xq"] = np.ascontiguousarray(xb[own].T)
        m["mask4"] = mask.astype(ml_dtypes.bfloat16)
        in_maps.append(m)
    return in_maps


def assemble_output(results):
    """results: list of 8 dicts with 'outT' [C, OWN] -> full [B, T, C] f32."""
    out = np.empty((B, T, C), dtype=np.float32)
    for c in range(8):
        b, p = c // 4, c % 4
        oT = results[c]["outT"].T  # [OWN, C] in slot order
        for s, BS in enumerate(SLOT_BOUNDS):
            blk = BS - 1 - p
            out[b, blk * P:(blk + 1) * P, :] = oT[s * P:(s + 1) * P, :]
    return out


_CACHED_NC = {}


def kernel(**inputs):
    from concourse.bass_utils import run_bass_kernel_spmd

    unit_gb = bool(
        np.all(np.asarray(inputs["ln1_g"]) == 1.0)
        and np.all(np.asarray(inputs["ln2_g"]) == 1.0)
        and np.all(np.asarray(inputs["ln1_b"]) == 0.0)
        and np.all(np.asarray(inputs["ln2_b"]) == 0.0))
    if unit_gb not in _CACHED_NC:
        _CACHED_NC[unit_gb] = build_program(unit_gb=unit_gb)
    in_maps = make_core_inputs(inputs)
    res = run_bass_kernel_spmd(_CACHED_NC[unit_gb], in_maps,
                               core_ids=list(range(8)))
    return assemble_output(res.results)


if __name__ == "__main__":
    nc = build_program()
    print("program built ok")



# revision 41
# speedup vs baseline: 1.0372x; 1.0372x over previous
"""Trainium2 Bass kernel for a single pre-norm transformer block.

Reference (B=2, T=2048, C=768, H=12, HD=64):
    x = x + causal_attn(LN1(x) @ W_qkv) @ W_attn_proj
    x = x + gelu(LN2(x) @ W_fc) @ W_mlp_proj

Sharding: 8 cores, zero collectives.  Core c = (batch b=c//4, rank p=c%4).
Causally balanced interleaved query blocks: core p owns the four 128-row
q-blocks {15-p, 11-p, 7-p, 3-p} of its batch, so every core's q-slots see
the uniform k-extents SLOT_BOUNDS=(16,12,8,4) chunks -- one SPMD program,
no per-core control flow, only ~18% causal overcompute.  Each core
recomputes LN1 + K/V for its whole batch (weights replicated; that is why
this problem is memory-regime).

Layout: feature-major activations [C on partitions, tokens free]; the
stationary matmul operand is always a natural weight tile, no activation
transposes.  Pipeline structure (v2):
  - LN1 is interleaved per 512-token group with the V and K projections,
    so the PE never drains while DVE normalizes: stats(qt+1) and V/K(qt)
    run behind normalize(qt).  All Q/K/V are computed before attention.
  - LN stats via matmul with an all-ones stationary tile (sums over
    features AND replicates to all partitions).
  - scores S^T[k,q]: lhsT = K fm chunk, rhs = Q fm slot; 8 k-chunks per
    PSUM bank pair, exp'd in one ScalarE op (bf16 out), multiplicative
    causal mask only on each slot's final 4-chunk group.
  - AV: lhsT = [V token-major | ones col] -> y + softmax denominator in
    one accumulation.  Head pairs are parity-packed: even head y rows
    0..63 + den row 64, odd head den row 63 + y rows 64..127, so both
    halves of y_sb are written by plain DVE ops (no partition-shift DMA).
  - softmax denominator reciprocal is broadcast across partitions with a
    1-row ones matmul on the PE (f32r, 1 cycle/row) instead of a DRAM
    roundtrip.
  - x DMAs ride the SP HWDGE queue, weight DMAs the Activation HWDGE
    queue, so neither blocks the other.
  - QKV/attention/MLP matmuls in bf16, LN arithmetic and residuals f32.
    Weights are host-pre-tiled so every weight load is one big DMA.
  - kernel() specializes away the LN scale/bias ops when gamma==1 and
    beta==0 (the spec fills) -- checked at runtime on the host.
"""

import sys

if "/opt/trn_rl_repo" not in sys.path:
    sys.path.insert(0, "/opt/trn_rl_repo")

import numpy as np

import concourse.bass as bass
import concourse.mybir as mybir
from concourse import bacc
import concourse.tile as tile

P = 128
B, T, C, H, HD = 2, 2048, 768, 12, 64
OWN = 512          # query rows owned by each core
NF = C // P        # 6 feature chunks
NQT = T // 512     # 4 column tiles over the 2048 tokens
NKT = T // P       # 16 key chunks
NMO_FC = (4 * C) // P  # 24
SLOT_BOUNDS = (16, 12, 8, 4)   # k-chunks processed per q-slot (128 q rows each)
EPS = 1e-5

f32 = mybir.dt.float32
f32r = mybir.dt.float32r
bf16 = mybir.dt.bfloat16
AFT = mybir.ActivationFunctionType
ALU = mybir.AluOpType

GELU_FUNC = AFT.Gelu  # dev sims patch an erf-gelu into bass_interp for this


def _r(ap):
    """View an f32 AP as float32r for full-rate PE matmuls."""
    return ap.bitcast(f32r)


def build_program(unit_gb=False):
    nc = bacc.Bacc()

    xT = nc.declare_dram_parameter("xT", [C, T], f32, False)[:]
    xq = nc.declare_dram_parameter("xq", [C, OWN], f32, False)[:]
    mask4 = nc.declare_dram_parameter("mask4", [P, 4, 4, P], bf16, False)[:]
    Wq_t = nc.declare_dram_parameter("Wq_t", [NF, P, NF, P], bf16, False)[:]
    Wk_t = nc.declare_dram_parameter("Wk_t", [NF, P, NF, P], bf16, False)[:]
    Wv_t = nc.declare_dram_parameter("Wv_t", [2, P, NF, 384], bf16, False)[:]
    Wap = nc.declare_dram_parameter("Wap", [C, C], bf16, False)[:]
    Wfc_t = nc.declare_dram_parameter("Wfc_t", [NF, P, 4, NF, P], bf16, False)[:]
    Wmp_t = nc.declare_dram_parameter("Wmp_t", [NF, P, NMO_FC, P], bf16, False)[:]
    g1 = nc.declare_dram_parameter("g1", [C], f32, False)[:]
    b1 = nc.declare_dram_parameter("b1", [C], f32, False)[:]
    g2 = nc.declare_dram_parameter("g2", [C], f32, False)[:]
    b2 = nc.declare_dram_parameter("b2", [C], f32, False)[:]
    outT = nc.declare_dram_parameter("outT", [C, OWN], f32, True)[:]

    # feature-chunked DRAM views: feature f = o*128 + p
    xT_r = xT.rearrange("(o p) t -> p o t", p=P)
    xq_r = xq.rearrange("(o p) t -> p o t", p=P)
    WapR = Wap.rearrange("(o p) m -> p o m", p=P)  # [128, 6, 768]
    outT_r = outT.rearrange("(o p) q -> p o q", p=P)

    with tile.TileContext(nc) as tc:
        _body(nc, tc, unit_gb, dict(
            xT_r=xT_r, xq_r=xq_r, Wq_t=Wq_t, Wk_t=Wk_t, Wv_t=Wv_t,
            Wfc_t=Wfc_t, Wmp_t=Wmp_t, WapR=WapR, mask4=mask4, outT_r=outT_r,
            g1=g1, b1=b1, g2=g2, b2=b2,
        ))
    nc.finalize()
    return nc


def _body(nc, tc, unit_gb, d):
    from contextlib import ExitStack

    with ExitStack() as ctx:
        def pool(name, bufs, space="SBUF", c=None):
            return (c or ctx).enter_context(
                tc.tile_pool(name=name, bufs=bufs, space=space))

        singles = pool("singles", 1)
        xpool = pool("xpool", 12)      # x chunks [P,512] f32 (3 qts in flight)
        sqf = pool("sqf", 3)           # bf16 staging + x^2 scratch for LN stats
        sqp = pool("sqp", 3)           # f32 normalize temps [P,512]
        statp = pool("statp", 5)       # LN stats [P,512] f32
        qpool = pool("qpool", 1)       # Q [P,6,512] bf16
        kpool = pool("kpool", 1)       # K [P,6,2048] bf16
        vpool = pool("vpool", 1)       # V [P,16,2,6,65] bf16
        maskp = pool("maskp", 1)
        wapp = pool("wapp", 1)
        x2pool = pool("x2pool", 6)     # x2 resident f32
        recp = pool("recp", 2)         # [1,512] f32

        ones_sb = singles.tile([P, P], bf16)
        nc.vector.memset(ones_sb[:], 1.0)
        ones1 = singles.tile([1, P], bf16)
        nc.vector.memset(ones1[:], 1.0)
        eps_sb = singles.tile([P, 1], f32)
        nc.vector.memset(eps_sb[:], EPS)
        # prime the ScalarE activation tables (Exp/Gelu/Square/Sqrt) during
        # the idle prologue so no LoadActFuncSet lands on a critical path
        warm = singles.tile([P, 1], f32)
        for fn in (AFT.Square, AFT.Sqrt, AFT.Exp, GELU_FUNC):
            nc.scalar.activation(out=warm[:], in_=eps_sb[:], func=fn)

        gb = {}
        if not unit_gb:
            for name in ("g1", "b1", "g2", "b2"):
                t = singles.tile([P, NF], f32, name=f"gb_{name}")
                nc.sync.dma_start(out=t[:], in_=d[name].rearrange("(o p) -> p o", p=P))
                gb[name] = t

        # V token-major, plus a ones column per head for the softmax
        # denominator row that rides along in the AV accumulation
        v_sb = vpool.tile([P, NKT, 2, NF, HD + 1], bf16)
        nc.vector.memset(v_sb[:, :, :, :, HD], 1.0)

        # ---------------- LN helpers (feature-major) ----------------
        # stats via bf16 matmuls with an all-ones stationary tile: sums over
        # the feature partitions AND replicates to all partitions.  The bf16
        # staging copies ride the otherwise-idle gpsimd engine.
        def stats_emit(srcs, ps_pool, copy_eng=None):
            s1 = ps_pool.tile([P, 512], f32, tag="st", name="s1")
            s2 = ps_pool.tile([P, 512], f32, tag="st", name="s2")
            for f in range(NF):
                xbf = sqf.tile([P, 512], bf16, tag="xbf", name=f"xbf{f}",
                               bufs=6)
                (copy_eng or nc.gpsimd).tensor_copy(out=xbf[:], in_=srcs[f])
                sq = sqf.tile([P, 512], bf16, tag="sqb", name=f"sq{f}",
                              bufs=3)
                nc.vector.tensor_tensor(sq[:], xbf[:], xbf[:], ALU.mult)
                nc.tensor.matmul(s1[:], ones_sb[:], xbf[:],
                                 start=(f == 0), stop=(f == NF - 1))
                nc.tensor.matmul(s2[:], ones_sb[:], sq[:],
                                 start=(f == 0), stop=(f == NF - 1))
            return s1, s2

        def chain_emit(s1, s2):
            mu = statp.tile([P, 512], f32, tag="stat", name="mu")
            nc.vector.tensor_scalar_mul(mu[:], s1[:], 1.0 / C)
            musq = statp.tile([P, 512], f32, tag="stat", name="musq")
            nc.scalar.activation(out=musq[:], in_=mu[:], func=AFT.Square)
            var = statp.tile([P, 512], f32, tag="stat", name="var")
            nc.vector.tensor_scalar(var[:], s2[:], 1.0 / C, None, ALU.mult)
            nc.vector.tensor_tensor(var[:], var[:], musq[:], ALU.subtract)
            std = statp.tile([P, 512], f32, tag="stat", name="std")
            nc.scalar.activation(out=std[:], in_=var[:], func=AFT.Sqrt,
                                 bias=eps_sb[:])
            rstd = statp.tile([P, 512], f32, tag="stat", name="rstd")
            nc.vector.reciprocal(rstd[:], std[:])
            return mu, rstd

        def ln_norm(srcs, mu, rstd, outs, g_sb, b_sb, n_dve=3):
            """outs[f] <- LN(srcs[f]); first n_dve chunks on DVE, rest on
            gpsimd (both engines stay off the PE critical path)."""
            for f in range(NF):
                eng = nc.vector if f < n_dve else nc.gpsimd
                t = sqp.tile([P, 512], f32, tag="sq", name=f"lnt{f}")
                eng.tensor_tensor(t[:], srcs[f], mu[:], ALU.subtract)
                if unit_gb:
                    eng.tensor_tensor(outs[f], t[:], rstd[:], ALU.mult)
                else:
                    eng.tensor_tensor(t[:], t[:], rstd[:], ALU.mult)
                    eng.tensor_scalar(outs[f], t[:],
                                      g_sb[:, f:f + 1], b_sb[:, f:f + 1],
                                      ALU.mult, ALU.add)

        # ============ phase 1+2: LN1 / V / K / Q pipelined per qt ============
        p2 = ExitStack()
        lnpool = pool("lnpool", 6, c=p2)    # xn1 resident bf16 [P, T+OWN]
        wqkv = pool("wqkv", 1, c=p2)
        ps_st = pool("ps_st", 3, space="PSUM", c=p2)
        ps_v = pool("ps_v", 2, space="PSUM", c=p2)
        ps_k = pool("ps_k", 3, space="PSUM", c=p2)

        # weight tiles: loaded on the Activation HWDGE queue, emitted in
        # need-order inside the qt loop so they never delay the x loads
        # (the DMA engines are a single shared resource)
        wv_sb = wqkv.tile([P, 2, NF, 384], bf16, name="wv_sb")
        wk_sb = wqkv.tile([P, NF, NF, P], bf16, name="wk_sb")
        wq_sb = wqkv.tile([P, NF, NF, P], bf16, name="wq_sb")
        wap_sb = wapp.tile([P, NF, C], bf16)
        mask_sb = maskp.tile([P, 4, 4, P], bf16)

        xn1 = [lnpool.tile([P, T + OWN], bf16, tag="ln1", name=f"xn1_{f}")
               for f in range(NF)]
        kt_sb = kpool.tile([P, NF, T], bf16)
        qt_sb = qpool.tile([P, NF, OWN], bf16)

        def load_x(qt):
            xs = []
            for f in range(NF):
                t = xpool.tile([P, 512], f32, tag="x", name=f"x_{f}_{qt}")
                if qt < NQT:
                    nc.sync.dma_start(out=t[:],
                                      in_=d["xT_r"][:, f, qt * 512:qt * 512 + 512])
                else:
                    nc.sync.dma_start(out=t[:], in_=d["xq_r"][:, f, :])
                xs.append(t)
            return xs

        # stats run one qt ahead of normalize, so the DVE/gpsimd normalize of
        # qt always hides under the PE's V/K matmuls of qt-1
        xq_all = [load_x(q) for q in range(2)]
        nc.scalar.dma_start(out=wv_sb[:],
                            in_=d["Wv_t"].rearrange("v p f c -> p v f c"))
        stq = [stats_emit([t[:] for t in xq_all[0]], ps_st)]
        for qt in range(NQT + 1):
            cs = slice(qt * 512, qt * 512 + 512)
            xs = xq_all[qt]
            mu, rstd = chain_emit(*stq[qt])
            if qt + 1 <= NQT:
                stq.append(stats_emit([t[:] for t in xq_all[qt + 1]], ps_st))
            ln_norm([t[:] for t in xs], mu, rstd,
                    [xn1[f][:, cs] for f in range(NF)],
                    gb.get("g1"), gb.get("b1"))
            if qt + 2 <= NQT:
                xq_all.append(load_x(qt + 2))
            if qt == 0:
                nc.scalar.dma_start(
                    out=wk_sb[:], in_=d["Wk_t"].rearrange("m p f c -> p m f c"))
            elif qt == 2:
                nc.scalar.dma_start(
                    out=wq_sb[:], in_=d["Wq_t"].rearrange("m p f c -> p m f c"))
            elif qt == 3:
                nc.scalar.dma_start(out=mask_sb[:], in_=d["mask4"])
            elif qt == NQT:
                nc.scalar.dma_start(out=wap_sb[:], in_=d["WapR"])

            if qt < NQT:
                # V for this qt's four 128-token chunks
                for kt in range(4 * qt, 4 * qt + 4):
                    ts = slice(kt * P, kt * P + P)
                    for vn in range(2):
                        ps = ps_v.tile([P, 512], f32, tag="v", name="vps")
                        for f in range(NF):
                            nc.tensor.matmul(ps[:, :384], xn1[f][:, ts],
                                             wv_sb[:, vn, f, :],
                                             start=(f == 0), stop=(f == NF - 1))
                        nc.scalar.activation(
                            out=v_sb[:, kt, vn, :, 0:HD],
                            in_=ps[:, :384].rearrange("p (j e) -> p j e", e=HD),
                            func=AFT.Copy)
                # K for this qt across all head-pairs
                for mo in range(NF):
                    ps = ps_k.tile([P, 512], f32, tag="k", name="kps")
                    for f in range(NF):
                        nc.tensor.matmul(ps[:], wk_sb[:, mo, f, :],
                                         xn1[f][:, cs],
                                         start=(f == 0), stop=(f == NF - 1))
                    if mo % 2 == 0:
                        nc.vector.tensor_copy(out=kt_sb[:, mo, cs], in_=ps[:])
                    else:
                        nc.scalar.activation(out=kt_sb[:, mo, cs], in_=ps[:],
                                             func=AFT.Copy)
            else:
                # Q for the owned 512 query rows
                for mo in range(NF):
                    ps = ps_k.tile([P, 512], f32, tag="k", name="qps")
                    for f in range(NF):
                        nc.tensor.matmul(ps[:], wq_sb[:, mo, f, :],
                                         xn1[f][:, T:T + OWN],
                                         start=(f == 0), stop=(f == NF - 1))
                    if mo % 2 == 0:
                        nc.vector.tensor_copy(out=qt_sb[:, mo, :], in_=ps[:])
                    else:
                        nc.scalar.activation(out=qt_sb[:, mo, :], in_=ps[:],
                                             func=AFT.Copy)
        x_qt4 = xq_all[NQT]
        p2.close()

        # ============ attention ============
        att = ExitStack()
        ypool = pool("ypool", 6, c=att)
        hbig = pool("hbig", 1, c=att)
        wfcp = pool("wfcp", 2, c=att)
        wmp_p = pool("wmp_p", 2, c=att)
        att_ps = ExitStack()
        apool = pool("apool", 4, c=att_ps)
        ytmp = pool("ytmp", 2, c=att_ps)
        recsp = pool("recsp", 2, c=att_ps)
        ps_s = pool("ps_s", 2, space="PSUM", c=att_ps)
        ps_y = pool("ps_y", 4, space="PSUM", c=att_ps)

        h_sb = hbig.tile([P, NMO_FC, OWN], bf16)

        # double-buffered MLP weight tiles; first two of each prefetched on
        # the (now idle) SP queue during attention, the rest at use sites
        wfc_t = {}

        def load_wfc(g4):
            w4 = wfcp.tile([P, 4, NF, P], bf16, tag="wfc", name=f"wfc{g4}",
                           bufs=2)
            nc.sync.dma_start(out=w4[:], in_=d["Wfc_t"][g4])
            wfc_t[g4] = w4

        wmp_t = {}

        def load_wmp(mo):
            w = wmp_p.tile([P, NMO_FC, P], bf16, tag="wmp", name=f"wmp{mo}",
                           bufs=2)
            nc.sync.dma_start(out=w[:], in_=d["Wmp_t"][mo])
            wmp_t[mo] = w

        y_sb = [ypool.tile([P, OWN], bf16, tag="y", name=f"y_{i}")
                for i in range(NF)]

        # the softmax-normalize tail of head h (broadcast reciprocal via a
        # 1-row ones matmul + multiply) is emitted into the NEXT head's first
        # exp-wait bubble so the recb matmul never stalls the PE queue
        pending = [None]

        def flush_pending():
            if pending[0] is not None:
                pending[0]()
                pending[0] = None

        # globally flattened (head, slot, group) stream with a one-group
        # score lookahead: scores of unit u+1 are emitted before AV of unit
        # u, so the exp chain on the Activation engine (the attention-phase
        # bottleneck) runs back-to-back with no per-group sync bubble
        units = []
        for mo in range(NF):
            for hh in (1, 0):
                h = 2 * mo + hh
                for s in range(4):
                    BS = SLOT_BOUNDS[s]
                    groups = [8] * (BS // 8) + ([4] if BS % 8 else [])
                    k0 = 0
                    for gi, gs in enumerate(groups):
                        units.append(dict(
                            mo=mo, hh=hh, h=h, po=hh * HD, s=s, gi=gi, gs=gs,
                            k0=k0, BS=BS, masked=(gi == len(groups) - 1),
                            first=(s == 0 and gi == 0),
                            last=(s == 3 and gi == len(groups) - 1)))
                        k0 += gs

        def emit_scores(u):
            q_s = qt_sb[u["po"]:u["po"] + HD, u["mo"],
                        u["s"] * P:(u["s"] + 1) * P]
            sp = ps_s.tile([P, 8, P], f32, tag="s",
                           name=f"sp{u['h']}_{u['s']}_{u['gi']}")
            for j in range(u["gs"]):
                kt = u["k0"] + j
                nc.tensor.matmul(
                    sp[:, j, :],
                    kt_sb[u["po"]:u["po"] + HD, u["mo"],
                          kt * P:(kt + 1) * P],
                    q_s, start=True, stop=True)
            return sp

        def finish_head(mo, hh, h, yp):
            rec = recp.tile([1, 512], bf16, tag="rec", name=f"rec{h}")
            with nc.allow_low_precision(reason="softmax denom reciprocal "
                                        "broadcast rides a bf16 ones-matmul"):
                nc.vector.reciprocal(rec[:], yp[HD:HD + 1, :])

            def go():
                recb = ps_y.tile([P, 512], f32, tag="y", name=f"recb{h}")
                nc.tensor.matmul(recb[:], ones1[:], rec[:],
                                 start=True, stop=True)
                recs = recsp.tile([HD, 512], f32, tag="recs",
                                  name=f"recs{h}")
                nc.vector.tensor_copy(out=recs[:], in_=recb[0:HD, :])
                if hh == 0:
                    nc.vector.tensor_tensor(y_sb[mo][0:HD, :], yp[0:HD, :],
                                            recs[:], ALU.mult)
                else:
                    yt = ytmp.tile([HD, 512], bf16, tag="yt", name=f"yt{h}")
                    nc.vector.tensor_tensor(yt[:], yp[0:HD, :],
                                            recs[:], ALU.mult)
                    nc.gpsimd.dma_start(out=y_sb[mo][HD:P, :], in_=yt[:])
            return go

        yp = None
        # two-unit score lookahead: scores stay ~2 exp-times ahead of the
        # Activation engine, so the post-exp mask / AV waits of one unit
        # never starve the exp chain (the attention-phase bottleneck)
        sps = [emit_scores(units[0]), emit_scores(units[1])]
        for i, u in enumerate(units):
            if u["first"]:
                yp = ps_y.tile([P, 512], f32, tag="y", name=f"yp{u['h']}")
            a_sb = apool.tile([P, 8, P], bf16, tag="a",
                              name=f"a{u['h']}_{u['s']}_{u['gi']}")
            nc.scalar.activation(out=a_sb[:, :u["gs"], :],
                                 in_=sps[i][:, :u["gs"], :], func=AFT.Exp,
                                 scale=float(1.0 / np.sqrt(HD)))
            if i + 2 < len(units):
                sps.append(emit_scores(units[i + 2]))
            if u["masked"]:
                gs = u["gs"]
                nc.vector.tensor_tensor(a_sb[:, gs - 4:gs, :],
                                        a_sb[:, gs - 4:gs, :],
                                        mask_sb[:, u["s"], :, :], ALU.mult)
            if u["first"]:
                flush_pending()
            yo = yp[0:HD + 1, u["s"] * P:(u["s"] + 1) * P]
            for j in range(u["gs"]):
                kt = u["k0"] + j
                nc.tensor.matmul(
                    yo, v_sb[:, kt, u["h"] // 6, u["h"] % 6, :],
                    a_sb[:, j, :],
                    start=(kt == 0),
                    stop=(kt == u["BS"] - 1))
            if u["last"]:
                pending[0] = finish_head(u["mo"], u["hh"], u["h"], yp)
                # prefetch the first MLP weight tiles on the SP queue while
                # attention runs (their ring slots are fresh)
                if u["mo"] == 0 and u["hh"] == 0:
                    load_wfc(0)
                    load_wfc(1)
                elif u["mo"] == 1 and u["hh"] == 0:
                    load_wmp(0)
                    load_wmp(1)
        flush_pending()
        att_ps.close()

        # ============ attn proj + residual ============
        mlp_ctx = ExitStack()
        xlnp2 = pool("xlnp2", 6, c=mlp_ctx)
        opool = pool("opool", 2, c=mlp_ctx)
        ps_mm = pool("ps_mm", 3, space="PSUM", c=mlp_ctx)

        # LN2 stats matmuls ride between the attn-proj matmuls so the PE
        # stays busy and mu2 is ready right after the last x2 chunk lands
        s1 = ps_mm.tile([P, 512], f32, tag="st", name="s21")
        s2 = ps_mm.tile([P, 512], f32, tag="st", name="s22")
        # accumulate attn-proj in head finish order (odd head of each pair
        # completes first; pair mo finishes before pair mo+1), so only the
        # final matmul of each output tile waits on the last head
        h2_order = [0, 1, 2, 3, 4, 5]
        x2 = []
        for f in range(NF):
            ps = ps_mm.tile([P, 512], f32, tag="mm", name="aps")
            for oi, h2 in enumerate(h2_order):
                nc.tensor.matmul(ps[:], wap_sb[:, h2, f * P:(f + 1) * P],
                                 y_sb[h2][:], start=(oi == 0), stop=(oi == NF - 1))
            x2t = x2pool.tile([P, OWN], f32, tag="x2", name=f"x2_{f}")
            nc.vector.tensor_tensor(x2t[:], ps[:], x_qt4[f][:], ALU.add)
            x2.append(x2t)
            xbf = sqf.tile([P, 512], bf16, tag="xbf", name=f"x2bf{f}",
                           bufs=6)
            nc.gpsimd.tensor_copy(out=xbf[:], in_=x2t[:])
            sq = sqf.tile([P, 512], bf16, tag="sqb", name=f"sq2_{f}", bufs=3)
            nc.vector.tensor_tensor(sq[:], xbf[:], xbf[:], ALU.mult)
            nc.tensor.matmul(s1[:], ones_sb[:], xbf[:],
                             start=(f == 0), stop=(f == NF - 1))
            nc.tensor.matmul(s2[:], ones_sb[:], sq[:],
                             start=(f == 0), stop=(f == NF - 1))

        # ============ LN2 + MLP ============
        mu2, rstd2 = chain_emit(s1, s2)
        xln2 = [xlnp2.tile([P, OWN], bf16, tag="ln2", name=f"xln2_{f}")
                for f in range(NF)]
        ln_norm([x2[f][:] for f in range(NF)], mu2, rstd2,
                [xln2[f][:] for f in range(NF)], gb.get("g2"), gb.get("b2"),
                n_dve=4)

        for g4 in range(NMO_FC // 4):
            if g4 + 2 < NMO_FC // 4:
                load_wfc(g4 + 2)
            for i in range(4):
                mo = g4 * 4 + i
                ps = ps_mm.tile([P, 512], f32, tag="mm", name="fps")
                for f in range(NF):
                    nc.tensor.matmul(ps[:], wfc_t[g4][:, i, f, :], xln2[f][:],
                                     start=(f == 0), stop=(f == NF - 1))
                nc.scalar.activation(out=h_sb[:, mo, :], in_=ps[:],
                                     func=GELU_FUNC)
        for mo in range(NF):
            if mo + 2 < NF:
                load_wmp(mo + 2)
            ps = ps_mm.tile([P, 512], f32, tag="mm", name="pps")
            for hc in range(NMO_FC):
                nc.tensor.matmul(ps[:], wmp_t[mo][:, hc, :], h_sb[:, hc, :],
                                 start=(hc == 0), stop=(hc == NMO_FC - 1))
            ot = opool.tile([P, OWN], f32, tag="o", name=f"o{mo}")
            nc.vector.tensor_tensor(ot[:], ps[:], x2[mo][:], ALU.add)
            nc.sync.dma_start(out=d["outT_r"][:, mo, :], in_=ot[:])
        mlp_ctx.close()
        att.close()


# ---------------------------------------------------------------------------
# host side
# ---------------------------------------------------------------------------

def make_core_inputs(inputs):
    """Build the 8 per-core input maps from the full-problem inputs."""
    import ml_dtypes

    x = np.asarray(inputs["x"], dtype=np.float32)
    bf = ml_dtypes.bfloat16

    def tile_w(w, n_mo, width):
        # [C_in, n_mo*width] -> [n_mo, 128, C_in//128, width]
        cin = w.shape[0]
        r = w.reshape(cin // P, P, n_mo, width)
        return np.ascontiguousarray(r.transpose(2, 1, 0, 3).astype(bf))

    Wqkv_f = np.asarray(inputs["W_qkv"], np.float32)
    Wfc_f = np.asarray(inputs["W_fc"], np.float32)
    Wmp_f = np.asarray(inputs["W_mlp_proj"], np.float32)
    wfc_t = tile_w(Wfc_f, 24, P).reshape(NF, 4, P, NF, P).transpose(0, 2, 1, 3, 4)
    full = {
        "Wq_t": tile_w(Wqkv_f[:, 0:C], NF, P),
        "Wk_t": tile_w(Wqkv_f[:, C:2 * C], NF, P),
        "Wv_t": tile_w(Wqkv_f[:, 2 * C:3 * C], 2, 384),
        "Wfc_t": np.ascontiguousarray(wfc_t),
        "Wmp_t": tile_w(Wmp_f, NF, P),
        "Wap": np.ascontiguousarray(np.asarray(inputs["W_attn_proj"], np.float32).astype(bf)),
        "g1": np.ascontiguousarray(np.asarray(inputs["ln1_g"], np.float32)),
        "b1": np.ascontiguousarray(np.asarray(inputs["ln1_b"], np.float32)),
        "g2": np.ascontiguousarray(np.asarray(inputs["ln2_g"], np.float32)),
        "b2": np.ascontiguousarray(np.asarray(inputs["ln2_b"], np.float32)),
    }
    in_maps = []
    for c in range(8):
        b, p = c // 4, c % 4
        blocks = [bs - 1 - p for bs in SLOT_BOUNDS]  # 128-row q-block indices
        xb = x[b]                                    # [T, C]
        own = np.concatenate([np.arange(bk * P, (bk + 1) * P) for bk in blocks])
        # mask4[kp, s, j, q] = keep for k-chunk (BS-4+j) of slot s vs q row
        mask = np.zeros((P, 4, 4, P), np.float32)
        kp = np.arange(P)[:, None]
        q = np.arange(P)[None, :]
        for s, BS in enumerate(SLOT_BOUNDS):
            E = BS - p          # real k-extent (chunks) for this core's block
            blk = BS - 1 - p    # q-block index
            for j in range(4):
                kc = BS - 4 + j
                if kc < E:
                    mask[:, s, j, :] = (kc * P + kp <= blk * P + q)
        m = dict(full)
        m["xT"] = np.ascontiguousarray(xb.T)
        m["xq"] = np.ascontiguousarray(xb[own].T)
        m["mask4"] = mask.astype(ml_dtypes.bfloat16)
        in_maps.append(m)
    return in_maps


def assemble_output(results):
    """results: list of 8 dicts with 'outT' [C, OWN] -> full [B, T, C] f32."""
    out = np.empty((B, T, C), dtype=np.float32)
    for c in range(8):
        b, p = c // 4, c % 4
        oT = results[c]["outT"].T  # [OWN, C] in slot order
        for s, BS in enumerate(SLOT_BOUNDS):
            blk = BS - 1 - p
            out[b, blk * P:(blk + 1) * P, :] = oT[s * P:(s + 1) * P, :]
    return out


_CACHED_NC = {}


def kernel(**inputs):
    from concourse.bass_utils import run_bass_kernel_spmd

    unit_gb = bool(
        np.all(np.asarray(inputs["ln1_g"]) == 1.0)
        and np.all(np.asarray(inputs["ln2_g"]) == 1.0)
        and np.all(np.asarray(inputs["ln1_b"]) == 0.0)
        and np.all(np.asarray(inputs["ln2_b"]) == 0.0))
    if unit_gb not in _CACHED_NC:
        _CACHED_NC[unit_gb] = build_program(unit_gb=unit_gb)
    in_maps = make_core_inputs(inputs)
    res = run_bass_kernel_spmd(_CACHED_NC[unit_gb], in_maps,
                               core_ids=list(range(8)))
    return assemble_output(res.results)


if __name__ == "__main__":
    nc = build_program()
    print("program built ok")
